# revision 7
# baseline (speedup 1.0000x reference)
"""Trainium2 Bass kernel for nn_Interpolator: zero-stuff upsample x8 + 128-tap FIR (SAME) + x8 gain.

Polyphase formulation: with m indexing 64-sample rows of x and n = 8*q' + r in [0, 512),
    y[512*m + n] = sum_{k=0}^{78} T4[k, m] * H4[k, n]
where T4[k, m] = x[64*m + k - 7] (zero-padded) and
    H4[k, 8*q'+r] = 8 * h[(7-r) + 8*(k-q')]  for 0 <= k-q' <= 15, else 0.

The T4 matrices are built on the HOST (numpy stride tricks) and shipped to the device
pre-transposed, so the device input path is 16 plain contiguous 81 KB loads on the
gpsimd (SWDGE) ring — no xbar DMA-transpose.  Per signal: 4 matmuls
lhsT=T4[0:79, 128t:+128], rhs=H4 [79, 512] fp16 -> PSUM fp32 [128, 512]; PSUM is
cast-copied to fp16 SBUF (alternating scalar/vector so the two PSUM-capable engines
split the 37 us of copy work) and stored with 256 KB DMAs on the sync ring, which
carries stores only.  y is fp16 on device; the host casts to fp32.
"""

import numpy as np

import concourse.bass as bass
import concourse.tile as tile
from concourse import bacc, mybir
from concourse.bass_utils import run_bass_kernel_spmd

B = 64
N = 32768
FACTOR = 8
NOUT = N * FACTOR  # 262144
N_CORES = 8
ROWS_PER_CORE = B // N_CORES  # 8
SIGS = 2 * ROWS_PER_CORE  # 16 signals per core (real rows then imag rows)
K = 79  # contraction window length
M = 512  # 64-sample blocks per signal
NPAD = 32832  # 7 leading zeros + N + 57 trailing zeros
TILES = 4  # out tiles per signal, each [128 m-rows, 512 samples]

_F16 = mybir.dt.float16
_F32 = mybir.dt.float32

_NC_CACHE = {}


def _build_nc():
    nc = bacc.Bacc(
        "TRN2",
        target_bir_lowering=False,
        debug=False,
        enable_asserts=False,
        num_devices=N_CORES,
    )
    GRP = 4  # signals per input load
    NGRP = SIGS // GRP
    xt = nc.dram_tensor("xt", [NGRP * K, GRP * M], _F16, kind="ExternalInput")
    h4 = nc.dram_tensor("h4", [K, 512], _F16, kind="ExternalInput")
    y = nc.dram_tensor("y", [SIGS, NOUT], _F16, kind="ExternalOutput")

    with tile.TileContext(nc) as tc:
        with (
            tc.tile_pool(name="consts", bufs=1) as consts,
            tc.tile_pool(name="t4pool", bufs=2) as t4pool,
            tc.tile_pool(name="opool", bufs=6) as opool,
            tc.tile_pool(name="po", bufs=3, space="PSUM") as po_pool,
            tc.tile_pool(name="warm", bufs=1, space="PSUM") as warm_pool,
        ):
            h4_sb = consts.tile([K, 512], _F16)
            nc.scalar.dma_start(out=h4_sb, in_=h4.ap())

            # PE warmup: ~8 matmuls on a zeroed dummy tile unthrottle HAM
            # (4/8 -> 8/8 clock) while the first input loads are in flight.
            dummy = consts.tile([K, 512], _F16)
            nc.gpsimd.memset(dummy, 0)
            warm_po = warm_pool.tile([128, 512], _F32)
            for _ in range(8):
                nc.tensor.matmul(
                    warm_po, dummy[0:K, 0:128], dummy[:, :], start=True, stop=True
                )

            grp_tiles = [None] * NGRP

            def load(g):
                T4 = t4pool.tile([K, GRP * M], _F16)
                nc.scalar.dma_start(
                    out=T4,
                    in_=bass.AP(
                        tensor=xt,
                        offset=g * K * GRP * M,
                        ap=[[GRP * M, K], [1, GRP * M]],
                    ),
                )
                grp_tiles[g] = T4

            ncopy = [0]

            def compute(sig):
                T4 = grp_tiles[sig // GRP]
                base = (sig % GRP) * M
                out_sb = opool.tile([128, 2048], _F16)
                for half in range(2):
                    po = po_pool.tile([128, 1024], _F32)
                    for s in range(2):
                        t = 2 * half + s
                        nc.tensor.matmul(
                            po[:, 512 * s : 512 * (s + 1)],
                            T4[0:K, base + 128 * t : base + 128 * (t + 1)],
                            h4_sb[:, :],
                            start=True,
                            stop=True,
                        )
                    # alternate copy engine by global counter: 15 scalar / 17 vector
                    # (scalar also carries the input loads)
                    c = ncopy[0]
                    ncopy[0] += 1
                    if c % 2 == 0 and c < 30:
                        nc.scalar.copy(
                            out=out_sb[:, 1024 * half : 1024 * (half + 1)], in_=po
                        )
                    else:
                        nc.vector.tensor_copy(
                            out=out_sb[:, 1024 * half : 1024 * (half + 1)], in_=po
                        )
                # partition i, free (t, n) -> y[sig, 65536t + 512i + n]
                nc.sync.dma_start(
                    out=bass.AP(
                        tensor=y,
                        offset=sig * NOUT,
                        ap=[[512, 128], [65536, 4], [1, 512]],
                    ),
                    in_=out_sb,
                )

            load(0)
            load(1)
            for sig in range(SIGS):
                if sig % GRP == 0 and sig // GRP + 2 < NGRP:
                    load(sig // GRP + 2)
                compute(sig)

    nc.compile()
    return nc


def _get_nc():
    if "nc" not in _NC_CACHE:
        _NC_CACHE["nc"] = _build_nc()
    return _NC_CACHE["nc"]


def _build_h4(h):
    h4 = np.zeros((K, 512), np.float32)
    qp = np.arange(64)
    for t in range(16):
        for r in range(8):
            h4[qp + t, 8 * qp + r] = FACTOR * h[(7 - r) + 8 * t]
    return h4


def _run(x_real, x_imag, fir_filter, trace=False):
    h4 = _build_h4(np.asarray(fir_filter, np.float32)).astype(np.float16)
    in_maps = []
    for c in range(N_CORES):
        rows = slice(c * ROWS_PER_CORE, (c + 1) * ROWS_PER_CORE)
        xp = np.zeros((SIGS, NPAD), np.float16)
        xp[:ROWS_PER_CORE, 7 : 7 + N] = x_real[rows]
        xp[ROWS_PER_CORE:, 7 : 7 + N] = x_imag[rows]
        # T4[sig, k, m] = xp[sig, 64*m + k] -- host-side transpose
        v = np.lib.stride_tricks.as_strided(
            xp,
            shape=(SIGS, K, M),
            strides=(xp.strides[1] * NPAD, xp.strides[1], 64 * xp.strides[1]),
        )
        # group 4 signals side by side: xt[g*K + k, s*M + m] = T4[4g+s][k, m]
        xt = np.ascontiguousarray(
            np.transpose(v.reshape(4, 4, K, M), (0, 2, 1, 3))
        ).reshape(4 * K, 4 * M)
        in_maps.append({"xt": xt, "h4": h4})
    nc = _get_nc()
    res = run_bass_kernel_spmd(nc, in_maps, core_ids=list(range(N_CORES)), trace=trace)
    out = np.empty((2, B, NOUT), np.float32)
    for c in range(N_CORES):
        yc = res.results[c]["y"]
        rows = slice(c * ROWS_PER_CORE, (c + 1) * ROWS_PER_CORE)
        out[0, rows] = yc[:ROWS_PER_CORE]
        out[1, rows] = yc[ROWS_PER_CORE:]
    return out, res


def kernel(x_real, x_imag, fir_filter, factor):
    assert int(factor) == FACTOR
    x_real = np.asarray(x_real, np.float32)
    x_imag = np.asarray(x_imag, np.float32)
    assert x_real.shape == (B, N) and x_imag.shape == (B, N)
    out, _ = _run(x_real, x_imag, fir_filter)
    return out


# revision 8
# speedup vs baseline: 1.5822x; 1.5822x over previous
"""Trainium2 Bass kernel for nn_Interpolator: zero-stuff upsample x8 + 128-tap FIR (SAME) + x8 gain.

Polyphase formulation with 128-sample input blocks: with m indexing 128-sample
blocks of x and n in [0, 1024),
    y[1024*m + n] = sum_{k=0}^{142} A[k, m] * H[k, n]
where A[k, m] = x[128*m + k - 7] (zero-padded) and
    H[k, n] = 8 * h[7 + 8k - n]  when 0 <= 7+8k-n < 128, else 0.
K=143 splits into a K=128 main matmul (lhsT = A column block) and a K=15 fixup
matmul from the next A column (only touches n in [896, 1024)).

The A matrices are built on the HOST (numpy stride tricks) and shipped
pre-transposed: the whole per-core input is ONE contiguous [128, 4112] fp16
load (128-partition DMAs spray across all 16 SDMA engines; odd-partition
shapes land on a single engine at ~22 GB/s).  Per signal-chunk: 3 matmuls into
PSUM [128, 1024] fp32, cast-copy to fp16 SBUF (alternating scalar/vector to
split the PSUM-read work over both PSUM-capable engines), then one fully
contiguous 256 KB store on the sync ring (2 KB per partition).  y is fp16 on
device; the host casts to fp32.  8 warmup matmuls on a zeroed tile unthrottle
the PE HAM clock gate during the initial load latency.
"""

import numpy as np

import concourse.bass as bass
import concourse.tile as tile
from concourse import bacc, mybir
from concourse.bass_utils import run_bass_kernel_spmd

B = 64
N = 32768
FACTOR = 8
NOUT = N * FACTOR  # 262144
N_CORES = 8
ROWS_PER_CORE = B // N_CORES  # 8
SIGS = 2 * ROWS_PER_CORE  # 16 signals per core (real rows then imag rows)
MP = N // 128  # 256 column blocks per signal
MCOL = MP + 1  # 257 columns of A per signal (one spill column for the fixup)
NPAD2 = 7 + N + 121  # 32896
KFIX = 15

_F16 = mybir.dt.float16
_F32 = mybir.dt.float32

_NC_CACHE = {}


def _build_nc():
    nc = bacc.Bacc(
        "TRN2",
        target_bir_lowering=False,
        debug=False,
        enable_asserts=False,
        num_devices=N_CORES,
    )
    xa = nc.dram_tensor("xa", [128, SIGS * MCOL], _F16, kind="ExternalInput")
    ha = nc.dram_tensor("ha", [128, 1024], _F16, kind="ExternalInput")
    hb = nc.dram_tensor("hb", [KFIX, 128], _F16, kind="ExternalInput")
    y = nc.dram_tensor("y", [SIGS, NOUT], _F16, kind="ExternalOutput")

    with tile.TileContext(nc) as tc:
        with (
            tc.tile_pool(name="consts", bufs=1) as consts,
            tc.tile_pool(name="opool", bufs=6) as opool,
            tc.tile_pool(name="po", bufs=3, space="PSUM") as po_pool,
            tc.tile_pool(name="warm", bufs=1, space="PSUM") as warm_pool,
        ):
            # whole per-core input in one contiguous 128-partition load
            xa_sb = consts.tile([128, SIGS * MCOL], _F16)
            nc.sync.dma_start(out=xa_sb, in_=xa.ap())
            ha_sb = consts.tile([128, 1024], _F16)
            nc.scalar.dma_start(out=ha_sb, in_=ha.ap())
            hb_sb = consts.tile([KFIX, 128], _F16)
            nc.scalar.dma_start(out=hb_sb, in_=hb.ap())

            # PE warmup: unthrottle HAM while the input load is in flight
            dummy = consts.tile([128, 512], _F16)
            nc.gpsimd.memset(dummy, 0)
            warm_po = warm_pool.tile([128, 512], _F32)
            for _ in range(8):
                nc.tensor.matmul(
                    warm_po, dummy[:, 0:128], dummy[:, :], start=True, stop=True
                )

            for it in range(2 * SIGS):
                sig, c = it // 2, it % 2
                col = sig * MCOL + 128 * c
                po = po_pool.tile([128, 1024], _F32)
                lhsT = xa_sb[0:128, col : col + 128]
                nc.tensor.matmul(
                    po[:, 0:512], lhsT, ha_sb[:, 0:512], start=True, stop=True
                )
                nc.tensor.matmul(
                    po[:, 512:1024], lhsT, ha_sb[:, 512:1024], start=True, stop=False
                )
                nc.tensor.matmul(
                    po[:, 896:1024],
                    xa_sb[0:KFIX, col + 1 : col + 129],
                    hb_sb[:, :],
                    start=False,
                    stop=True,
                )
                out_sb = opool.tile([128, 1024], _F16)
                if it % 2 == 0:
                    nc.scalar.copy(out=out_sb, in_=po)
                else:
                    nc.vector.tensor_copy(out=out_sb, in_=po)
                # fully contiguous 256 KB store: y[sig, 131072c + 1024i + j]
                nc.sync.dma_start(
                    out=bass.AP(
                        tensor=y,
                        offset=sig * NOUT + c * 131072,
                        ap=[[1024, 128], [1, 1024]],
                    ),
                    in_=out_sb,
                )

    nc.compile()
    return nc


def _get_nc():
    if "nc" not in _NC_CACHE:
        _NC_CACHE["nc"] = _build_nc()
    return _NC_CACHE["nc"]


def _build_h(h):
    """H[k, n] = 8 h[7 + 8k - n] when 0 <= 7+8k-n < 128; returns (Ha, Hb)."""
    H = np.zeros((143, 1024), np.float32)
    k = np.arange(143)[:, None]
    n = np.arange(1024)[None, :]
    i = 7 + 8 * k - n
    m = (i >= 0) & (i < 128)
    H[m] = FACTOR * np.asarray(h, np.float32)[i[m]]
    return H[0:128].astype(np.float16), H[128:143, 896:1024].astype(np.float16)


def _run(x_real, x_imag, fir_filter, trace=False):
    ha, hb = _build_h(np.asarray(fir_filter, np.float32))
    in_maps = []
    for c in range(N_CORES):
        rows = slice(c * ROWS_PER_CORE, (c + 1) * ROWS_PER_CORE)
        xp = np.zeros((SIGS, NPAD2), np.float16)
        xp[:ROWS_PER_CORE, 7 : 7 + N] = x_real[rows]
        xp[ROWS_PER_CORE:, 7 : 7 + N] = x_imag[rows]
        # A[sig, k, m] = xp[sig, 128*m + k] -> device layout [k, sig*MCOL + m]
        v = np.lib.stride_tricks.as_strided(
            xp,
            shape=(SIGS, 128, MCOL),
            strides=(xp.strides[1] * NPAD2, xp.strides[1], 128 * xp.strides[1]),
        )
        xa = np.ascontiguousarray(np.transpose(v, (1, 0, 2))).reshape(128, SIGS * MCOL)
        in_maps.append({"xa": xa, "ha": ha, "hb": hb})
    nc = _get_nc()
    res = run_bass_kernel_spmd(nc, in_maps, core_ids=list(range(N_CORES)), trace=trace)
    out = np.empty((2, B, NOUT), np.float32)
    for c in range(N_CORES):
        yc = res.results[c]["y"]
        rows = slice(c * ROWS_PER_CORE, (c + 1) * ROWS_PER_CORE)
        out[0, rows] = yc[:ROWS_PER_CORE]
        out[1, rows] = yc[ROWS_PER_CORE:]
    return out, res


def kernel(x_real, x_imag, fir_filter, factor):
    assert int(factor) == FACTOR
    x_real = np.asarray(x_real, np.float32)
    x_imag = np.asarray(x_imag, np.float32)
    assert x_real.shape == (B, N) and x_imag.shape == (B, N)
    out, _ = _run(x_real, x_imag, fir_filter)
    return out


# revision 9
# speedup vs baseline: 1.8553x; 1.1726x over previous
"""Trainium2 Bass kernel for nn_Interpolator: zero-stuff upsample x8 + 128-tap FIR (SAME) + x8 gain.

Polyphase formulation with 128-sample input blocks: with m indexing 128-sample
blocks of x and n in [0, 1024),
    y[1024*m + n] = sum_{k=0}^{142} A[k, m] * H[k, n]
where A[k, m] = x[128*m + k - 7] (zero-padded) and
    H[k, n] = 8 * h[7 + 8k - n]  when 0 <= 7+8k-n < 128, else 0.
K=143 splits into a K=128 main matmul (lhsT = A column block) and a K=15 fixup
matmul from the next A column (only touches n in [896, 1024)).

The A matrices are built on the HOST (numpy stride tricks) and shipped
pre-transposed: the whole per-core input is ONE contiguous [128, 4112] fp16
load (128-partition DMAs spray across all 16 SDMA engines; odd-partition
shapes land on a single engine at ~22 GB/s).  Per signal-chunk: 3 matmuls into
PSUM [128, 1024] fp32, cast-copy to fp16 SBUF (alternating scalar/vector to
split the PSUM-read work over both PSUM-capable engines), then one fully
contiguous 256 KB store on the sync ring (2 KB per partition).  y is fp16 on
device; the host casts to fp32.  8 warmup matmuls on a zeroed tile unthrottle
the PE HAM clock gate during the initial load latency.
"""

import numpy as np

import concourse.bass as bass
import concourse.tile as tile
from concourse import bacc, mybir
from concourse.bass_utils import run_bass_kernel_spmd

B = 64
N = 32768
FACTOR = 8
NOUT = N * FACTOR  # 262144
N_CORES = 8
ROWS_PER_CORE = B // N_CORES  # 8
SIGS = 2 * ROWS_PER_CORE  # 16 signals per core (real rows then imag rows)
MP = N // 128  # 256 column blocks per signal
MCOL = MP + 1  # 257 columns of A per signal (one spill column for the fixup)
NPAD2 = 7 + N + 121  # 32896
KFIX = 15

_F16 = mybir.dt.float16
_F32 = mybir.dt.float32

_NC_CACHE = {}


def _build_nc():
    nc = bacc.Bacc(
        "TRN2",
        target_bir_lowering=False,
        debug=False,
        enable_asserts=False,
        num_devices=N_CORES,
    )
    xa = nc.dram_tensor("xa", [128, SIGS * MCOL], _F16, kind="ExternalInput")
    ha = nc.dram_tensor("ha", [128, 1024], _F16, kind="ExternalInput")
    hb = nc.dram_tensor("hb", [KFIX, 128], _F16, kind="ExternalInput")
    y = nc.dram_tensor("y", [SIGS, NOUT], _F16, kind="ExternalOutput")

    with tile.TileContext(nc) as tc:
        with (
            tc.tile_pool(name="consts", bufs=1) as consts,
            tc.tile_pool(name="opool", bufs=6) as opool,
            tc.tile_pool(name="po", bufs=3, space="PSUM") as po_pool,
            tc.tile_pool(name="warm", bufs=1, space="PSUM") as warm_pool,
        ):
            # whole per-core input in one contiguous 128-partition load
            xa_sb = consts.tile([128, SIGS * MCOL], _F16)
            nc.sync.dma_start(out=xa_sb, in_=xa.ap())
            ha_sb = consts.tile([128, 1024], _F16)
            nc.scalar.dma_start(out=ha_sb, in_=ha.ap())
            hb_sb = consts.tile([KFIX, 128], _F16)
            nc.scalar.dma_start(out=hb_sb, in_=hb.ap())

            # PE warmup: unthrottle HAM while the input load is in flight
            dummy = consts.tile([128, 512], _F16)
            nc.gpsimd.memset(dummy, 0)
            warm_po = warm_pool.tile([128, 512], _F32)
            for _ in range(24):
                nc.tensor.matmul(
                    warm_po, dummy[:, 0:128], dummy[:, :], start=True, stop=True
                )

            for it in range(2 * SIGS):
                sig, c = it // 2, it % 2
                col = sig * MCOL + 128 * c
                po = po_pool.tile([128, 1024], _F32)
                lhsT = xa_sb[0:128, col : col + 128]
                nc.tensor.matmul(
                    po[:, 0:512], lhsT, ha_sb[:, 0:512], start=True, stop=True
                )
                nc.tensor.matmul(
                    po[:, 512:1024], lhsT, ha_sb[:, 512:1024], start=True, stop=False
                )
                nc.tensor.matmul(
                    po[:, 896:1024],
                    xa_sb[0:KFIX, col + 1 : col + 129],
                    hb_sb[:, :],
                    start=False,
                    stop=True,
                )
                out_sb = opool.tile([128, 1024], _F16)
                if it % 2 == 0:
                    nc.scalar.copy(out=out_sb, in_=po)
                else:
                    nc.vector.tensor_copy(out=out_sb, in_=po)
                # fully contiguous 256 KB store: y[sig, 131072c + 1024i + j]
                nc.sync.dma_start(
                    out=bass.AP(
                        tensor=y,
                        offset=sig * NOUT + c * 131072,
                        ap=[[1024, 128], [1, 1024]],
                    ),
                    in_=out_sb,
                )

    nc.compile()
    return nc


def _get_nc():
    if "nc" not in _NC_CACHE:
        _NC_CACHE["nc"] = _build_nc()
    return _NC_CACHE["nc"]


def _build_h(h):
    """H[k, n] = 8 h[7 + 8k - n] when 0 <= 7+8k-n < 128; returns (Ha, Hb)."""
    H = np.zeros((143, 1024), np.float32)
    k = np.arange(143)[:, None]
    n = np.arange(1024)[None, :]
    i = 7 + 8 * k - n
    m = (i >= 0) & (i < 128)
    H[m] = FACTOR * np.asarray(h, np.float32)[i[m]]
    return H[0:128].astype(np.float16), H[128:143, 896:1024].astype(np.float16)


def _run(x_real, x_imag, fir_filter, trace=False):
    ha, hb = _build_h(np.asarray(fir_filter, np.float32))
    in_maps = []
    for c in range(N_CORES):
        rows = slice(c * ROWS_PER_CORE, (c + 1) * ROWS_PER_CORE)
        xp = np.zeros((SIGS, NPAD2), np.float16)
        xp[:ROWS_PER_CORE, 7 : 7 + N] = x_real[rows]
        xp[ROWS_PER_CORE:, 7 : 7 + N] = x_imag[rows]
        # A[sig, k, m] = xp[sig, 128*m + k] -> device layout [k, sig*MCOL + m]
        v = np.lib.stride_tricks.as_strided(
            xp,
            shape=(SIGS, 128, MCOL),
            strides=(xp.strides[1] * NPAD2, xp.strides[1], 128 * xp.strides[1]),
        )
        xa = np.ascontiguousarray(np.transpose(v, (1, 0, 2))).reshape(128, SIGS * MCOL)
        in_maps.append({"xa": xa, "ha": ha, "hb": hb})
    nc = _get_nc()
    res = run_bass_kernel_spmd(nc, in_maps, core_ids=list(range(N_CORES)), trace=trace)
    out = np.empty((2, B, NOUT), np.float32)
    for c in range(N_CORES):
        yc = res.results[c]["y"]
        rows = slice(c * ROWS_PER_CORE, (c + 1) * ROWS_PER_CORE)
        out[0, rows] = yc[:ROWS_PER_CORE]
        out[1, rows] = yc[ROWS_PER_CORE:]
    return out, res


def kernel(x_real, x_imag, fir_filter, factor):
    assert int(factor) == FACTOR
    x_real = np.asarray(x_real, np.float32)
    x_imag = np.asarray(x_imag, np.float32)
    assert x_real.shape == (B, N) and x_imag.shape == (B, N)
    out, _ = _run(x_real, x_imag, fir_filter)
    return out


# revision 14
# speedup vs baseline: 1.9737x; 1.0638x over previous
"""Trainium2 Bass kernel for nn_Interpolator: zero-stuff upsample x8 + 128-tap FIR (SAME) + x8 gain.

Polyphase formulation with 128-sample input blocks: with m indexing 128-sample
blocks of x and n in [0, 1024),
    y[1024*m + n] = sum_{k=0}^{142} A[k, m] * H[k, n]
where A[k, m] = x[128*m + k - 7] (zero-padded) and
    H[k, n] = 8 * h[7 + 8k - n]  when 0 <= 7+8k-n < 128, else 0.
K=143 splits into a K=128 main matmul (lhsT = A column block) and a K=15 fixup
matmul from the next A column (only touches n in [896, 1024)).

The A matrices are built on the HOST (numpy stride tricks) and shipped
pre-transposed: the whole per-core input is ONE contiguous [128, 4112] fp16
load (128-partition DMAs spray across all 16 SDMA engines; odd-partition
shapes land on a single engine at ~22 GB/s).  Per signal-chunk: 3 matmuls into
PSUM [128, 1024] fp32, cast-copy to fp16 SBUF (alternating scalar/vector to
split the PSUM-read work over both PSUM-capable engines), then one fully
contiguous 256 KB store on the sync ring (2 KB per partition).  y is fp16 on
device; the host casts to fp32.  8 warmup matmuls on a zeroed tile unthrottle
the PE HAM clock gate during the initial load latency.
"""

import numpy as np

import concourse.bass as bass
import concourse.tile as tile
from concourse import bacc, mybir
from concourse.bass_utils import run_bass_kernel_spmd

B = 64
N = 32768
FACTOR = 8
NOUT = N * FACTOR  # 262144
N_CORES = 8
ROWS_PER_CORE = B // N_CORES  # 8
SIGS = 2 * ROWS_PER_CORE  # 16 signals per core (real rows then imag rows)
MP = N // 128  # 256 column blocks per signal
MCOL = MP + 1  # 257 columns of A per signal (one spill column for the fixup)
NPAD2 = 7 + N + 121  # 32896
KFIX = 15

_F16 = mybir.dt.float16
_F32 = mybir.dt.float32

_NC_CACHE = {}


def _build_nc():
    nc = bacc.Bacc(
        "TRN2",
        target_bir_lowering=False,
        debug=False,
        enable_asserts=False,
        num_devices=N_CORES,
    )
    xa = nc.dram_tensor("xa", [128, SIGS * MCOL], _F16, kind="ExternalInput")
    ha = nc.dram_tensor("ha", [128, 1024], _F16, kind="ExternalInput")
    hb = nc.dram_tensor("hb", [KFIX, 128], _F16, kind="ExternalInput")
    y = nc.dram_tensor("y", [SIGS, NOUT], _F16, kind="ExternalOutput")

    with tile.TileContext(nc) as tc:
        with (
            tc.tile_pool(name="consts", bufs=1) as consts,
            tc.tile_pool(name="opool", bufs=6) as opool,
            tc.tile_pool(name="po", bufs=4, space="PSUM") as po_pool,
        ):
            # whole per-core input in one contiguous 128-partition load
            xa_sb = consts.tile([128, SIGS * MCOL], _F16)
            nc.sync.dma_start(out=xa_sb, in_=xa.ap())
            ha_sb = consts.tile([128, 1024], _F16)
            nc.scalar.dma_start(out=ha_sb, in_=ha.ap())
            hb_sb = consts.tile([KFIX, 128], _F16)
            nc.scalar.dma_start(out=hb_sb, in_=hb.ap())

            # PE warmup: a contiguous burst of back-to-back matmuls unthrottles
            # the HAM clock gate (4/8 -> 8/8).  The pipelined real stream has
            # micro-gaps that keep resetting the activity window, so the burst
            # must be gap-free and span >= ~2 windows (~7 us) BEFORE real work.
            dummy = consts.tile([128, 512], _F16)
            nc.gpsimd.memset(dummy, 0)
            for _ in range(16):
                warm_po = po_pool.tile([128, 1024], _F32, tag="po")
                nc.tensor.matmul(
                    warm_po[:, 0:512], dummy[:, 0:128], dummy[:, :], start=True, stop=True
                )

            for it in range(2 * SIGS):
                sig, c = it // 2, it % 2
                col = sig * MCOL + 128 * c
                po = po_pool.tile([128, 1024], _F32, tag="po")
                lhsT = xa_sb[0:128, col : col + 128]
                nc.tensor.matmul(
                    po[:, 0:512], lhsT, ha_sb[:, 0:512], start=True, stop=True
                )
                nc.tensor.matmul(
                    po[:, 512:1024], lhsT, ha_sb[:, 512:1024], start=True, stop=False
                )
                nc.tensor.matmul(
                    po[:, 896:1024],
                    xa_sb[0:KFIX, col + 1 : col + 129],
                    hb_sb[:, :],
                    start=False,
                    stop=True,
                )
                out_sb = opool.tile([128, 1024], _F16)
                # 17 scalar / 15 vector balances engine busy time
                # (ACTIVATE ~1114 ns vs CAST ~1223 ns per copy)
                if it % 2 == 0 or it == 31:
                    nc.scalar.copy(out=out_sb, in_=po)
                else:
                    nc.vector.tensor_copy(out=out_sb, in_=po)
                # fully contiguous 256 KB store: y[sig, 131072c + 1024i + j]
                nc.sync.dma_start(
                    out=bass.AP(
                        tensor=y,
                        offset=sig * NOUT + c * 131072,
                        ap=[[1024, 128], [1, 1024]],
                    ),
                    in_=out_sb,
                )

    nc.compile()
    return nc


def _get_nc():
    if "nc" not in _NC_CACHE:
        _NC_CACHE["nc"] = _build_nc()
    return _NC_CACHE["nc"]


def _build_h(h):
    """H[k, n] = 8 h[7 + 8k - n] when 0 <= 7+8k-n < 128; returns (Ha, Hb)."""
    H = np.zeros((143, 1024), np.float32)
    k = np.arange(143)[:, None]
    n = np.arange(1024)[None, :]
    i = 7 + 8 * k - n
    m = (i >= 0) & (i < 128)
    H[m] = FACTOR * np.asarray(h, np.float32)[i[m]]
    return H[0:128].astype(np.float16), H[128:143, 896:1024].astype(np.float16)


def _run(x_real, x_imag, fir_filter, trace=False):
    ha, hb = _build_h(np.asarray(fir_filter, np.float32))
    in_maps = []
    for c in range(N_CORES):
        rows = slice(c * ROWS_PER_CORE, (c + 1) * ROWS_PER_CORE)
        xp = np.zeros((SIGS, NPAD2), np.float16)
        xp[:ROWS_PER_CORE, 7 : 7 + N] = x_real[rows]
        xp[ROWS_PER_CORE:, 7 : 7 + N] = x_imag[rows]
        # A[sig, k, m] = xp[sig, 128*m + k] -> device layout [k, sig*MCOL + m]
        v = np.lib.stride_tricks.as_strided(
            xp,
            shape=(SIGS, 128, MCOL),
            strides=(xp.strides[1] * NPAD2, xp.strides[1], 128 * xp.strides[1]),
        )
        xa = np.ascontiguousarray(np.transpose(v, (1, 0, 2))).reshape(128, SIGS * MCOL)
        in_maps.append({"xa": xa, "ha": ha, "hb": hb})
    nc = _get_nc()
    res = run_bass_kernel_spmd(nc, in_maps, core_ids=list(range(N_CORES)), trace=trace)
    out = np.empty((2, B, NOUT), np.float32)
    for c in range(N_CORES):
        yc = res.results[c]["y"]
        rows = slice(c * ROWS_PER_CORE, (c + 1) * ROWS_PER_CORE)
        out[0, rows] = yc[:ROWS_PER_CORE]
        out[1, rows] = yc[ROWS_PER_CORE:]
    return out, res


def kernel(x_real, x_imag, fir_filter, factor):
    assert int(factor) == FACTOR
    x_real = np.asarray(x_real, np.float32)
    x_imag = np.asarray(x_imag, np.float32)
    assert x_real.shape == (B, N) and x_imag.shape == (B, N)
    out, _ = _run(x_real, x_imag, fir_filter)
    return out
